# revision 1
# baseline (speedup 1.0000x reference)
"""2-layer GCN (GEMM -> COO SpMM -> ReLU -> GEMM -> SpMM) on 8 trn2 NeuronCores.

Design (row-sharded, transpose-free):
  - Core m owns node rows [m*RPC, (m+1)*RPC); padded to RPAD=NB*128 rows on
    device (pad rows never referenced by gathers; dropped on host).
  - GEMM1: Z1 = X @ W1 + b1 per-core (node-major), bf16, AllGather -> Z1_full.
  - SpMM: per 128-row block, edges sorted by col, split into 4 col-quartile
    segments so gather indices fit int16 relative to a QBASE-row view of
    Z_full.  dma_gather (non-transpose) emits slot-major [128 slots, 128 f]
    chunks == matmul lhsT directly.  S[slot, row] = (iota==row_local)*val is
    one DVE tensor_scalar per chunk.  PE accumulates
    out^T[feats, rows] += G^T @ S in PSUM over a block's Q*CAP_CH chunks.
  - out^T feature-major == lhsT layout for the next GEMM (no transposes).
  - Output written feature-major [128, RPAD] f32; host transposes + trims.

SPMD: one program for 8 cores; fixed slot layout (CAP_CH chunks of 128 per
(block, quartile), padded with idx=0/val=0), per-core data varies only in
input tensors.  DMA-instruction count before fan-in points is minimized:
HW limits sync-waits per instruction and Tile round-robins each DMA onto
one of 8 HWDGE semaphore lanes.
"""

import sys

import numpy as np
import ml_dtypes

_TRN_REPO = "/opt/trn_rl_repo"
if _TRN_REPO not in sys.path:
    sys.path.insert(0, _TRN_REPO)

import concourse.bass as bass
import concourse.tile as tile
from concourse import bacc, mybir
from concourse.bass_utils import run_bass_kernel_spmd

BF16 = mybir.dt.bfloat16
F32 = mybir.dt.float32
I16 = mybir.dt.int16


class Cfg:
    def __init__(self, n_nodes, in_size, hidden, out_size,
                 cap_ch=5, group_blocks=5):
        self.M = 8
        self.NN = n_nodes
        self.IN = in_size
        self.HID = hidden
        self.OUT = out_size
        assert n_nodes % self.M == 0
        self.RPC = n_nodes // self.M          # real rows per core
        self.BL = 128
        self.NB = (self.RPC + 127) // 128
        self.RPAD = self.NB * 128             # padded rows per core
        self.NNP = self.M * self.RPAD         # padded global nodes
        self.Q = 4
        assert self.NNP % self.Q == 0
        self.QBASE = self.NNP // self.Q
        assert self.QBASE <= 32768
        self.CAP_CH = cap_ch
        self.CAP = cap_ch * 128
        self.GB = group_blocks
        self.KIN = in_size // 128
        assert in_size % 128 == 0 and hidden == 128 and out_size == 128


FULL = Cfg(100000, 256, 128, 128)


def build_plan(cfg, row, col, vals):
    row = np.asarray(row).astype(np.int64)
    col = np.asarray(col).astype(np.int64)
    vals = np.asarray(vals).astype(np.float32)
    # remap cols into padded node space
    colp = (col // cfg.RPC) * cfg.RPAD + (col % cfg.RPC)

    # adaptive per-(block, quartile) capacity: scan max segment first
    need = 0
    for m in range(cfg.M):
        sel = (row // cfg.RPC) == m
        er0 = row[sel] - m * cfg.RPC
        key = (er0 // cfg.BL) * cfg.Q + colp[sel] // cfg.QBASE
        if key.size:
            need = max(need, int(np.bincount(key.astype(np.int64)).max()))
    cap_ch = max(cfg.CAP_CH, -(-need // 128))
    if cap_ch != cfg.CAP_CH:
        cfg.CAP_CH = cap_ch
        cfg.CAP = cap_ch * 128

    groups = [list(range(g, min(g + cfg.GB, cfg.NB)))
              for g in range(0, cfg.NB, cfg.GB)]
    slot_off = {}
    insts = []  # (q, slot_offset, n_slots) per (group, quartile)
    off = 0
    for blist in groups:
        for q in range(cfg.Q):
            ioff = off
            for b in blist:
                slot_off[(b, q)] = off
                off += cfg.CAP
            insts.append((q, ioff, off - ioff))
    nslot = off
    nchunk = nslot // 128

    per_core = []
    max_seg = 0
    for m in range(cfg.M):
        sel = (row // cfg.RPC) == m
        er = (row[sel] - m * cfg.RPC).astype(np.int64)
        ec = colp[sel]
        ev = vals[sel]
        blk = er // cfg.BL
        order = np.lexsort((ec, blk))
        er, ec, ev, blk = er[order], ec[order], ev[order], blk[order]

        idx16 = np.zeros(nslot, dtype=np.int16)
        rloc = np.zeros(nslot, dtype=np.float32)
        sval = np.zeros(nslot, dtype=np.float32)

        bstart = np.searchsorted(blk, np.arange(cfg.NB + 1))
        for b in range(cfg.NB):
            i0, i1 = bstart[b], bstart[b + 1]
            ecb = ec[i0:i1]
            qsplit = np.searchsorted(ecb, np.arange(cfg.Q + 1) * cfg.QBASE)
            for q in range(cfg.Q):
                j0, j1 = i0 + qsplit[q], i0 + qsplit[q + 1]
                n = j1 - j0
                max_seg = max(max_seg, n)
                if n > cfg.CAP:
                    raise RuntimeError(
                        f"segment overflow core {m} blk {b} q {q}: "
                        f"{n} > {cfg.CAP}")
                so = slot_off[(b, q)]
                idx16[so:so + n] = (ec[j0:j1] - q * cfg.QBASE).astype(np.int16)
                rloc[so:so + n] = (er[j0:j1] - b * cfg.BL).astype(np.float32)
                sval[so:so + n] = ev[j0:j1]

        idx_w = np.tile(idx16.reshape(-1, 16).T, (8, 1))
        rloc_w = rloc.reshape(nchunk, 128).T.astype(np.float32)
        sval_w = sval.reshape(nchunk, 128).T.astype(np.float32)
        per_core.append(dict(idx=np.ascontiguousarray(idx_w),
                             rloc=np.ascontiguousarray(rloc_w),
                             sval=np.ascontiguousarray(sval_w)))
    return groups, insts, slot_off, nslot, nchunk, per_core, max_seg


def build_program(cfg, groups, insts, slot_off, nslot, nchunk):
    nc = bacc.Bacc("TRN2", target_bir_lowering=False, debug=False,
                   num_devices=cfg.M)

    xt_d = nc.dram_tensor("xt", [cfg.IN, cfg.RPAD], BF16, kind="ExternalInput")
    wcols = cfg.KIN * 128 + 128 + 4 * 128
    wpack_d = nc.dram_tensor("wpack", [128, wcols], BF16, kind="ExternalInput")
    idx_d = nc.dram_tensor("idx", [128, nslot // 16], I16, kind="ExternalInput")
    fcols = 2 * nchunk
    fpack_d = nc.dram_tensor("fpack", [128, fcols], F32, kind="ExternalInput")
    out_d = nc.dram_tensor("out", [128, cfg.RPAD], F32, kind="ExternalOutput")

    z1_loc = nc.dram_tensor("z1_loc", [cfg.RPAD, cfg.HID], BF16)
    z1_full = nc.dram_tensor("z1_full", [cfg.NNP, cfg.HID], BF16)
    z2_loc = nc.dram_tensor("z2_loc", [cfg.RPAD, cfg.OUT], BF16)
    z2_full = nc.dram_tensor("z2_full", [cfg.NNP, cfg.OUT], BF16)

    rg = [list(range(cfg.M))]

    with tile.TileContext(nc) as tc:
        from contextlib import ExitStack
        with ExitStack() as ctx:
            const = ctx.enter_context(tc.tile_pool(name="const", bufs=1))
            xt_pool = ctx.enter_context(tc.tile_pool(name="xt", bufs=8))
            gbuf_pool = ctx.enter_context(tc.tile_pool(name="gbuf", bufs=2))
            s_pool = ctx.enter_context(tc.tile_pool(name="sm", bufs=4))
            ot_pool = ctx.enter_context(tc.tile_pool(name="ot", bufs=8))
            rt_pool = ctx.enter_context(tc.tile_pool(name="rt", bufs=1))
            psum_g = ctx.enter_context(
                tc.tile_pool(name="psum_g", bufs=2, space="PSUM"))
            psum_s = ctx.enter_context(
                tc.tile_pool(name="psum_s", bufs=6, space="PSUM"))

            # ---- resident constants (3 packed loads: bounded sem fan-in) ----
            wpack_sb = const.tile([128, wcols], BF16, tag="wpack",
                                  name="wpacksb")
            nc.sync.dma_start(wpack_sb[:], wpack_d[:, :])
            w1_sb = [wpack_sb[:, k * 128:(k + 1) * 128]
                     for k in range(cfg.KIN)]
            o = cfg.KIN * 128
            w2_sb = wpack_sb[:, o:o + 128]
            b1_sb = wpack_sb[0:1, o + 128:o + 256]
            b2_sb = wpack_sb[0:1, o + 256:o + 384]
            ones_sb = wpack_sb[0:1, o + 384:o + 512]
            iota_sb = wpack_sb[:, o + 512:o + 640]
            idx_sb = const.tile([128, nslot // 16], I16, tag="idx",
                                name="idxsb")
            nc.sync.dma_start(idx_sb[:], idx_d[:, :])
            fpack_sb = const.tile([128, fcols], F32, tag="fpack",
                                  name="fpacksb")
            nc.sync.dma_start(fpack_sb[:], fpack_d[:, :])
            rloc_sb = fpack_sb[:, 0:nchunk]
            sval_sb = fpack_sb[:, nchunk:2 * nchunk]
            rt_sb = rt_pool.tile([128, cfg.RPAD], BF16, tag="rt", name="rtsb")
            zs_sb = rt_pool.tile([128, cfg.RPAD], BF16, tag="zs", name="zssb")

            def gemm(lhsT_of, w_list, bias, zdst):
                """Z[t] = lhsT_t.T @ W + 1.b ; evac into zs_sb; one DMA out."""
                for t in range(cfg.NB):
                    ps = psum_g.tile([128, 128], F32, tag="gemm_ps", name="ps")
                    for k, (lt, wk) in enumerate(zip(lhsT_of(t), w_list)):
                        nc.tensor.matmul(ps[:], lt, wk,
                                         start=(k == 0), stop=False,
                                         skip_group_check=True)
                    nc.tensor.matmul(ps[:], ones_sb, bias,
                                     start=False, stop=True,
                                     skip_group_check=True)
                    nc.scalar.copy(zs_sb[:, t * 128:(t + 1) * 128], ps[:])
                nc.gpsimd.dma_start(
                    zdst.rearrange("(t p) f -> p t f", p=128)[:, :, :],
                    zs_sb.rearrange("p (t f) -> p t f", f=128)[:, :, :])

            # ---- GEMM1 ----
            def x_lhsT(t):
                tiles = []
                for k in range(cfg.KIN):
                    xt = xt_pool.tile([128, 128], BF16, tag="xt", name="xt")
                    nc.sync.dma_start(
                        xt[:], xt_d[k * 128:(k + 1) * 128,
                                    t * 128:(t + 1) * 128])
                    tiles.append(xt[:])
                return tiles

            gemm(x_lhsT, w1_sb, b1_sb, z1_loc)
            nc.gpsimd.collective_compute(
                "AllGather", mybir.AluOpType.bypass, replica_groups=rg,
                ins=[z1_loc[:, :]], outs=[z1_full[:, :]])

            # ---- SpMM ----
            def spmm(z_full, layer):
                for gi, blist in enumerate(groups):
                    nbl = len(blist)
                    ptiles = [psum_s.tile([128, 128], F32, tag="spmm_ps",
                                          name="spmm_ps")
                              for _ in range(nbl)]

                    def pview(bi):
                        return ptiles[bi][:, :]

                    for q in range(cfg.Q):
                        qi, ioff, n = insts[gi * cfg.Q + q]
                        assert qi == q
                        gb3 = gbuf_pool.tile(
                            [128, cfg.GB * cfg.CAP // 128, 128], BF16,
                            tag="gbuf", name="gbuf")
                        gb = gb3.rearrange("p c f -> p (c f)")
                        # SWDGE ring holds ~1024 descriptors; split gathers
                        o = 0
                        while o < n:
                            nj = min(1024, n - o)
                            nc.gpsimd.dma_gather(
                                out_ap=gb3[:, o // 128:(o + nj) // 128, :],
                                in_ap=z_full[q * cfg.QBASE:
                                             (q + 1) * cfg.QBASE, :],
                                idxs_ap=idx_sb[:, (ioff + o) // 16:
                                               (ioff + o + nj) // 16],
                                num_idxs=nj, num_idxs_reg=nj,
                                elem_size=cfg.HID,
                            )
                            o += nj
                        for bi, b in enumerate(blist):
                            for c in range(cfg.CAP_CH):
                                so = slot_off[(b, q)] - ioff + c * 128
                                cg = (slot_off[(b, q)] + c * 128) // 128
                                s = s_pool.tile([128, 128], BF16, tag="s",
                                                name="s")
                                nc.vector.tensor_scalar(
                                    s[:], iota_sb,
                                    rloc_sb[:, cg:cg + 1],
                                    sval_sb[:, cg:cg + 1],
                                    mybir.AluOpType.is_equal,
                                    mybir.AluOpType.mult)
                                nc.tensor.matmul(
                                    pview(bi), gb[:, so:so + 128], s[:],
                                    start=(q == 0 and c == 0),
                                    stop=(q == cfg.Q - 1 and
                                          c == cfg.CAP_CH - 1),
                                    skip_group_check=True)
                    for pi, pt in enumerate(ptiles):
                        b0 = blist[pi]
                        nw = 128
                        r0 = b0 * 128
                        if layer == 1:
                            nc.scalar.activation(
                                rt_sb[:, r0:r0 + nw], pt[:, :nw],
                                mybir.ActivationFunctionType.Relu)
                        else:
                            ot = ot_pool.tile([128, 512], F32, tag="ot",
                                              name="ot")
                            nc.scalar.copy(ot[:, :nw], pt[:, :nw])
                            nc.sync.dma_start(out_d[:, r0:r0 + nw],
                                              ot[:, :nw])

            spmm(z1_full, 1)

            # ---- GEMM2 ----
            def rt_lhsT(t):
                return [rt_sb[:, t * 128:(t + 1) * 128]]

            gemm(rt_lhsT, [w2_sb], b2_sb, z2_loc)
            nc.gpsimd.collective_compute(
                "AllGather", mybir.AluOpType.bypass, replica_groups=rg,
                ins=[z2_loc[:, :]], outs=[z2_full[:, :]])

            spmm(z2_full, 2)

    nc.compile()
    return nc


def _prep_inputs(cfg, X, W1, b1, W2, b2, per_core, nchunk):
    bf = ml_dtypes.bfloat16
    wcols = cfg.KIN * 128 + 128 + 4 * 128
    wpack = np.zeros((128, wcols), dtype=np.float32)
    for k in range(cfg.KIN):
        wpack[:, k * 128:(k + 1) * 128] = np.asarray(W1)[k * 128:(k + 1) * 128]
    o = cfg.KIN * 128
    wpack[:, o:o + 128] = np.asarray(W2)
    wpack[0, o + 128:o + 256] = np.asarray(b1)
    wpack[0, o + 256:o + 384] = np.asarray(b2)
    wpack[0, o + 384:o + 512] = 1.0
    wpack[:, o + 512:o + 640] = np.arange(128, dtype=np.float32)[None, :]
    wpack = wpack.astype(bf)

    X = np.asarray(X).astype(np.float32)
    in_maps = []
    for m in range(cfg.M):
        xs = np.zeros((cfg.IN, cfg.RPAD), dtype=np.float32)
        xs[:, :cfg.RPC] = X[m * cfg.RPC:(m + 1) * cfg.RPC].T
        fpack = np.zeros((128, 2 * nchunk), dtype=np.float32)
        fpack[:, :nchunk] = per_core[m]["rloc"]
        fpack[:, nchunk:] = per_core[m]["sval"]
        in_maps.append(dict(
            xt=np.ascontiguousarray(xs.astype(bf)), wpack=wpack,
            idx=per_core[m]["idx"], fpack=fpack))
    return in_maps


def run(cfg, X, W1, b1, W2, b2, vals, row, col, trace=False):
    groups, insts, slot_off, nslot, nchunk, per_core, max_seg = \
        build_plan(cfg, row, col, vals)
    nc = build_program(cfg, groups, insts, slot_off, nslot, nchunk)
    in_maps = _prep_inputs(cfg, X, W1, b1, W2, b2, per_core, nchunk)
    res = run_bass_kernel_spmd(nc, in_maps, list(range(cfg.M)), trace=trace)
    outs = [np.asarray(res.results[m]["out"]).T[:cfg.RPC]
            for m in range(cfg.M)]
    out = np.concatenate(outs, axis=0).astype(np.float32)
    return out, res


def kernel(X, W1, b1, W2, b2, vals, row, col):
    out, _ = run(FULL, X, W1, b1, W2, b2, vals, row, col)
    return out



# revision 17
# speedup vs baseline: 1.9250x; 1.9250x over previous
"""2-layer GCN (Linear -> SpMM -> ReLU -> Linear -> SpMM) on 8 trn2 cores.

Reformulation vs the classic GEMM-first pipeline:
  layer1:  A @ (X W1 + b1) == (A @ X) W1 + rowsum(A) b1^T
    Every core holds the FULL input X (padded bf16 copy "xg"), so the
    SpMM gather for layer 1 reads X directly -- NO AllGather for layer 1,
    and the gather elements are 512B (256 bf16 feats), dodging the <512B
    DMA half-bandwidth penalty.  rowsum(A) is precomputed on host.
  layer2:  classic: Z2 = relu(H1) W2 + b2 per-core, AllGather Z2
    (Shared-output HBM collective), gather 256B rows.

SpMM per 128-row block: edges sorted by col, split into Q=4 col-quartile
segments (idx fits int16 vs a QBASE-row window).  dma_gather emits
slot-major [128 slots, F] chunks == matmul lhsT.  S[slot, row] =
(iota==rloc)*val via one fused tensor_scalar per chunk, split between
DVE (majority) and Pool.  PE accumulates out^T[feat, rows] += G^T @ S
in PSUM across a block's segments.

Gathers are batched 7 blocks x CAP per instruction (large SWDGE ring)
and issued from Pool; idx data is streamed per batch instead of held
resident.  All non-gather DMA is issued from SP/Act HWDGE.  The single
collective is Pool-issued (API constraint) with a Shared output tensor.
"""

import sys

import numpy as np
import ml_dtypes

_TRN_REPO = "/opt/trn_rl_repo"
if _TRN_REPO not in sys.path:
    sys.path.insert(0, _TRN_REPO)

import concourse.bass as bass
import concourse.tile as tile
from concourse import bacc, mybir
from concourse.bass_utils import run_bass_kernel_spmd

BF16 = mybir.dt.bfloat16
F32 = mybir.dt.float32
I16 = mybir.dt.int16


class Cfg:
    def __init__(self, n_nodes, in_size, hidden, out_size,
                 cap_ch=5, group_blocks=6, dve_share=8, iters=1):
        self.M = 8
        self.NN = n_nodes
        self.IN = in_size
        self.HID = hidden
        self.OUT = out_size
        assert n_nodes % self.M == 0
        self.RPC = n_nodes // self.M          # real rows per core
        self.BL = 128
        self.NB = (self.RPC + 127) // 128
        self.RPAD = self.NB * 128             # padded rows per core
        self.NNP = self.M * self.RPAD         # padded global nodes
        self.Q = 4
        assert self.NNP % self.Q == 0
        self.QBASE = self.NNP // self.Q
        assert self.QBASE <= 32768
        self.CAP_CH = cap_ch
        self.CAP = cap_ch * 128
        self.GB = group_blocks
        self.KIN = in_size // 128
        assert in_size % 128 == 0 and hidden == 128 and out_size == 128
        # S-build engine split: chunk c -> DVE unless c % dve_share == 0
        self.DVE_SHARE = dve_share
        self.ITERS = iters


FULL = Cfg(100000, 256, 128, 128)


def build_plan(cfg, row, col, vals):
    row = np.asarray(row).astype(np.int64)
    col = np.asarray(col).astype(np.int64)
    vals = np.asarray(vals).astype(np.float32)
    # remap cols into padded node space
    colp = (col // cfg.RPC) * cfg.RPAD + (col % cfg.RPC)

    # adaptive per-(block, quartile) capacity: scan max segment first
    need = 0
    for m in range(cfg.M):
        sel = (row // cfg.RPC) == m
        er0 = row[sel] - m * cfg.RPC
        key = (er0 // cfg.BL) * cfg.Q + colp[sel] // cfg.QBASE
        if key.size:
            need = max(need, int(np.bincount(key.astype(np.int64)).max()))
    cap_ch = max(cfg.CAP_CH, -(-need // 128))
    if cap_ch != cfg.CAP_CH:
        cfg.CAP_CH = cap_ch
        cfg.CAP = cap_ch * 128

    groups = [list(range(g, min(g + cfg.GB, cfg.NB)))
              for g in range(0, cfg.NB, cfg.GB)]
    slot_off = {}
    insts = []  # (q, slot_offset, n_slots) per (group, quartile)
    off = 0
    for blist in groups:
        for q in range(cfg.Q):
            ioff = off
            for b in blist:
                slot_off[(b, q)] = off
                off += cfg.CAP
            insts.append((q, ioff, off - ioff))
    nslot = off
    nchunk = nslot // 128

    per_core = []
    max_seg = 0
    for m in range(cfg.M):
        sel = (row // cfg.RPC) == m
        er = (row[sel] - m * cfg.RPC).astype(np.int64)
        ec = colp[sel]
        ev = vals[sel]
        blk = er // cfg.BL
        order = np.lexsort((ec, blk))
        er, ec, ev, blk = er[order], ec[order], ev[order], blk[order]

        idx16 = np.zeros(nslot, dtype=np.int16)
        rloc = np.zeros(nslot, dtype=np.float32)
        sval = np.zeros(nslot, dtype=np.float32)

        bstart = np.searchsorted(blk, np.arange(cfg.NB + 1))
        for b in range(cfg.NB):
            i0, i1 = bstart[b], bstart[b + 1]
            ecb = ec[i0:i1]
            qsplit = np.searchsorted(ecb, np.arange(cfg.Q + 1) * cfg.QBASE)
            for q in range(cfg.Q):
                j0, j1 = i0 + qsplit[q], i0 + qsplit[q + 1]
                n = j1 - j0
                max_seg = max(max_seg, n)
                if n > cfg.CAP:
                    raise RuntimeError(
                        f"segment overflow core {m} blk {b} q {q}: "
                        f"{n} > {cfg.CAP}")
                so = slot_off[(b, q)]
                idx16[so:so + n] = (ec[j0:j1] - q * cfg.QBASE).astype(np.int16)
                rloc[so:so + n] = (er[j0:j1] - b * cfg.BL).astype(np.float32)
                sval[so:so + n] = ev[j0:j1]

        idx_w = np.tile(idx16.reshape(-1, 16).T, (8, 1))
        rloc_w = rloc.reshape(nchunk, 128).T.astype(np.float32)
        sval_w = sval.reshape(nchunk, 128).T.astype(np.float32)

        # per-core rowsum of A (for the layer-1 bias term), padded
        rs = np.zeros(cfg.RPAD, dtype=np.float64)
        np.add.at(rs, er, ev.astype(np.float64))
        per_core.append(dict(idx=np.ascontiguousarray(idx_w),
                             rloc=np.ascontiguousarray(rloc_w),
                             sval=np.ascontiguousarray(sval_w),
                             rowsum=rs.astype(np.float32)))
    return groups, insts, slot_off, nslot, nchunk, per_core, max_seg


def build_program(cfg, groups, insts, slot_off, nslot, nchunk):
    import os
    _RING = int(os.environ.get("RING", str(1 << 14)))
    _SHARED = bool(int(os.environ.get("SHARED", "1")))
    nc = bacc.Bacc("TRN2", target_bir_lowering=False, debug=False,
                   num_devices=cfg.M,
                   dynamic_dma_scratch_size=_RING)
    GMAX = _RING // 16 // 2  # gather split size: half the SWDGE ring

    xg_d = nc.dram_tensor("xg", [cfg.NNP, cfg.IN], BF16, kind="ExternalInput")
    wcols = cfg.KIN * 128 + 128 + 4 * 128
    wpack_d = nc.dram_tensor("wpack", [128, wcols], BF16, kind="ExternalInput")
    rs_d = nc.dram_tensor("rs", [1, cfg.RPAD], BF16, kind="ExternalInput")
    idx_d = nc.dram_tensor("idx", [128, nslot // 16], I16, kind="ExternalInput")
    fcols = 2 * nchunk
    fpack_d = nc.dram_tensor("fpack", [128, fcols], F32, kind="ExternalInput")
    out_d = nc.dram_tensor("out", [128, cfg.RPAD], F32, kind="ExternalOutput")

    z2_loc = nc.dram_tensor("z2_loc", [cfg.RPAD, cfg.HID], BF16)
    if _SHARED:
        z2_full = nc.dram_tensor("z2_full", [cfg.NNP, cfg.HID], BF16,
                                 addr_space="Shared")
    else:
        z2_full = nc.dram_tensor("z2_full", [cfg.NNP, cfg.HID], BF16)

    rg = [list(range(cfg.M))]
    NG = len(groups)

    with tile.TileContext(nc) as tc:
        from contextlib import ExitStack
        with ExitStack() as ctx:
            const = ctx.enter_context(tc.tile_pool(name="const", bufs=1))
            idx_pool = ctx.enter_context(tc.tile_pool(name="idxp", bufs=3))
            g1_pool = ctx.enter_context(tc.tile_pool(name="g1", bufs=2))
            g2_pool = ctx.enter_context(tc.tile_pool(name="g2", bufs=2))
            s_pool = ctx.enter_context(tc.tile_pool(name="sm", bufs=8))
            h1_pool = ctx.enter_context(tc.tile_pool(name="h1", bufs=4))
            ot_pool = ctx.enter_context(tc.tile_pool(name="ot", bufs=8))
            rt_pool = ctx.enter_context(tc.tile_pool(name="rt", bufs=1))
            ps = ctx.enter_context(
                tc.tile_pool(name="ps", bufs=cfg.GB, space="PSUM"))
            ps_g = ctx.enter_context(
                tc.tile_pool(name="ps_g", bufs=8 - cfg.GB, space="PSUM"))

            # ---- resident constants ----
            wpack_sb = const.tile([128, wcols], BF16, tag="wpack",
                                  name="wpacksb")
            nc.sync.dma_start(wpack_sb[:], wpack_d[:, :])
            w1_sb = [wpack_sb[:, k * 128:(k + 1) * 128]
                     for k in range(cfg.KIN)]
            o = cfg.KIN * 128
            w2_sb = wpack_sb[:, o:o + 128]
            b1_sb = wpack_sb[0:1, o + 128:o + 256]
            b2_sb = wpack_sb[0:1, o + 256:o + 384]
            ones_sb = wpack_sb[0:1, o + 384:o + 512]
            iota_sb = wpack_sb[:, o + 512:o + 640]
            rs_sb = const.tile([1, cfg.RPAD], BF16, tag="rs", name="rssb")
            nc.sync.dma_start(rs_sb[:], rs_d[:, :])
            fpack_sb = const.tile([128, fcols], F32, tag="fpack",
                                  name="fpacksb")
            nc.sync.dma_start(fpack_sb[:], fpack_d[:, :])
            rloc_sb = fpack_sb[:, 0:nchunk]
            sval_sb = fpack_sb[:, nchunk:2 * nchunk]
            rt_sb = rt_pool.tile([128, cfg.RPAD], BF16, tag="rt", name="rtsb")
            zs_sb = rt_pool.tile([128, cfg.RPAD], BF16, tag="zs", name="zssb")

            def make_s(cg):
                """S[slot, row] = (iota == rloc)*val for global chunk cg."""
                s = s_pool.tile([128, 128], BF16, tag="s", name="s")
                eng = nc.vector if (cg % cfg.DVE_SHARE) else nc.gpsimd
                eng.tensor_scalar(
                    s[:], iota_sb,
                    rloc_sb[:, cg:cg + 1],
                    sval_sb[:, cg:cg + 1],
                    mybir.AluOpType.is_equal,
                    mybir.AluOpType.mult)
                return s

            def gather(src, gi, q, nf, pool):
                """One batched gather for (group gi, quartile q) from src."""
                qi, ioff, n = insts[gi * cfg.Q + q]
                assert qi == q
                gb3 = pool.tile([128, n // 128, nf], BF16,
                                tag=f"gb{nf}", name="gb")
                it = idx_pool.tile([128, n // 16], I16, tag="idx", name="it")
                nc.sync.dma_start(it[:], idx_d[:, ioff // 16:(ioff + n) // 16])
                o = 0
                while o < n:
                    nj = min(GMAX, n - o)
                    nc.gpsimd.dma_gather(
                        out_ap=gb3[:, o // 128:(o + nj) // 128, :],
                        in_ap=src[q * cfg.QBASE:(q + 1) * cfg.QBASE, :],
                        idxs_ap=it[:, o // 16:(o + nj) // 16],
                        num_idxs=nj, num_idxs_reg=nj,
                        elem_size=nf,
                    )
                    o += nj
                return gb3

            # =========================== layer 1 ===========================
            # H1^T[f, r] (f in 2 tiles of 128) accumulated per block over
            # quartiles; then Z1^T = W1^T H1 + b1 rs^T, relu -> rt_sb.
            for it_ in range(cfg.ITERS):
                for gi, blist in enumerate(groups):
                    nbl = len(blist)
                    # one PSUM bank per block: [f0-tile | f1-tile] packed
                    pab = [ps.tile([128, 256], F32, tag="ps", name="pab")
                           for _ in range(nbl)]
                    for q in range(cfg.Q):
                        gb3 = gather(xg_d, gi, q, cfg.IN, g1_pool)
                        gb = gb3.rearrange("p c f -> p (c f)")
                        ioff = insts[gi * cfg.Q + q][1]
                        for bi, b in enumerate(blist):
                            for c in range(cfg.CAP_CH):
                                so = slot_off[(b, q)] - ioff + c * 128
                                cg = (slot_off[(b, q)] + c * 128) // 128
                                s = make_s(cg)
                                st = (q == 0 and c == 0)
                                sp = (q == cfg.Q - 1 and c == cfg.CAP_CH - 1)
                                co = (so // 128) * cfg.IN
                                # one start per PSUM bank: it marks the whole
                                # 2KB zero-region pending; the B-half's first
                                # write auto-overwrites via pending-zero
                                nc.tensor.matmul(
                                    pab[bi][:, 0:128],
                                    gb[:, co:co + 128], s[:],
                                    start=st, stop=sp, skip_group_check=True)
                                nc.tensor.matmul(
                                    pab[bi][:, 128:256],
                                    gb[:, co + 128:co + 256], s[:],
                                    start=False, stop=sp,
                                    skip_group_check=True)
                    # per-block: evac H1^T, GEMM1 (Z1^T), relu -> rt_sb
                    for bi, b in enumerate(blist):
                        h1 = h1_pool.tile([128, 256], BF16, tag="h1",
                                          name="h1")
                        nc.scalar.copy(h1[:], pab[bi][:, :])
                        r0 = b * 128
                        # pz | pz2 packed into one PSUM bank
                        pzc = ps_g.tile([128, 256], F32, tag="pz", name="pz")
                        pz = pzc[:, 0:128]
                        pz2 = pzc[:, 128:256]
                        nc.tensor.matmul(pz, w1_sb[0], h1[:, 0:128],
                                         start=True, stop=False,
                                         skip_group_check=True)
                        nc.tensor.matmul(pz, w1_sb[1], h1[:, 128:256],
                                         start=False, stop=False,
                                         skip_group_check=True)
                        nc.tensor.matmul(pz, b1_sb,
                                         rs_sb[0:1, r0:r0 + 128],
                                         start=False, stop=True,
                                         skip_group_check=True)
                        nc.scalar.activation(
                            rt_sb[:, r0:r0 + 128], pz,
                            mybir.ActivationFunctionType.Relu)
                        # GEMM2: Z2[r, h2] = H[r, :] W2 + b2
                        # (start=False: pz's start already marked this bank
                        # pending-zero, so the first write here overwrites)
                        nc.tensor.matmul(pz2, rt_sb[:, r0:r0 + 128],
                                         w2_sb, start=False, stop=False,
                                         skip_group_check=True)
                        nc.tensor.matmul(pz2, ones_sb, b2_sb,
                                         start=False, stop=True,
                                         skip_group_check=True)
                        nc.scalar.copy(zs_sb[:, r0:r0 + 128], pz2)
                    # stream this group's rows of Z2 out to DRAM
                    nc.sync.dma_start(
                        z2_loc.rearrange("(t p) f -> p t f", p=128)[
                            :, blist[0]:blist[0] + nbl, :],
                        zs_sb.rearrange("p (t f) -> p t f", f=128)[
                            :, blist[0]:blist[0] + nbl, :])

                # ---- AllGather Z2 (Shared output) ----
                nc.gpsimd.collective_compute(
                    "AllGather", mybir.AluOpType.bypass, replica_groups=rg,
                    ins=[z2_loc[:, :]], outs=[z2_full[:, :]])

                # =========================== layer 2 =======================
                for gi, blist in enumerate(groups):
                    nbl = len(blist)
                    pot = [ps.tile([128, 256], F32, tag="ps", name="po")
                           for _ in range(nbl)]
                    po = [t[:, 0:128] for t in pot]
                    for q in range(cfg.Q):
                        gb3 = gather(z2_full, gi, q, cfg.HID, g2_pool)
                        gb = gb3.rearrange("p c f -> p (c f)")
                        ioff = insts[gi * cfg.Q + q][1]
                        for bi, b in enumerate(blist):
                            for c in range(cfg.CAP_CH):
                                so = slot_off[(b, q)] - ioff + c * 128
                                cg = (slot_off[(b, q)] + c * 128) // 128
                                s = make_s(cg)
                                nc.tensor.matmul(
                                    po[bi], gb[:, so:so + 128], s[:],
                                    start=(q == 0 and c == 0),
                                    stop=(q == cfg.Q - 1 and
                                          c == cfg.CAP_CH - 1),
                                    skip_group_check=True)
                    for bi, b in enumerate(blist):
                        r0 = b * 128
                        ot = ot_pool.tile([128, 128], F32, tag="ot",
                                          name="ot")
                        nc.scalar.copy(ot[:], po[bi])
                        nc.sync.dma_start(out_d[:, r0:r0 + 128], ot[:])

    nc.compile()
    return nc


def _prep_inputs(cfg, X, W1, b1, W2, b2, per_core, nchunk):
    bf = ml_dtypes.bfloat16
    wcols = cfg.KIN * 128 + 128 + 4 * 128
    wpack = np.zeros((128, wcols), dtype=np.float32)
    for k in range(cfg.KIN):
        wpack[:, k * 128:(k + 1) * 128] = np.asarray(W1)[k * 128:(k + 1) * 128]
    o = cfg.KIN * 128
    wpack[:, o:o + 128] = np.asarray(W2)
    wpack[0, o + 128:o + 256] = np.asarray(b1)
    wpack[0, o + 256:o + 384] = np.asarray(b2)
    wpack[0, o + 384:o + 512] = 1.0
    wpack[:, o + 512:o + 640] = np.arange(128, dtype=np.float32)[None, :]
    wpack = wpack.astype(bf)

    X = np.asarray(X).astype(np.float32)
    xg = np.zeros((cfg.NNP, cfg.IN), dtype=bf)
    for m in range(cfg.M):
        xg[m * cfg.RPAD:m * cfg.RPAD + cfg.RPC] = \
            X[m * cfg.RPC:(m + 1) * cfg.RPC].astype(bf)

    in_maps = []
    for m in range(cfg.M):
        fpack = np.zeros((128, 2 * nchunk), dtype=np.float32)
        fpack[:, :nchunk] = per_core[m]["rloc"]
        fpack[:, nchunk:] = per_core[m]["sval"]
        rs = per_core[m]["rowsum"].astype(bf).reshape(1, cfg.RPAD)
        in_maps.append(dict(
            xg=xg, wpack=wpack, rs=rs,
            idx=per_core[m]["idx"], fpack=fpack))
    return in_maps


def run(cfg, X, W1, b1, W2, b2, vals, row, col, trace=False):
    groups, insts, slot_off, nslot, nchunk, per_core, max_seg = \
        build_plan(cfg, row, col, vals)
    nc = build_program(cfg, groups, insts, slot_off, nslot, nchunk)
    in_maps = _prep_inputs(cfg, X, W1, b1, W2, b2, per_core, nchunk)
    res = run_bass_kernel_spmd(nc, in_maps, list(range(cfg.M)), trace=trace)
    outs = [np.asarray(res.results[m]["out"]).T[:cfg.RPC]
            for m in range(cfg.M)]
    out = np.concatenate(outs, axis=0).astype(np.float32)
    return out, res


def kernel(X, W1, b1, W2, b2, vals, row, col):
    out, _ = run(FULL, X, W1, b1, W2, b2, vals, row, col)
    return out
